# revision 19
# baseline (speedup 1.0000x reference)
"""Trainium2 Bass kernel for a Swin transformer block (DRSwinNet).

Sharding: data-parallel over batch -- 32 images / 8 cores = 4 images each,
params replicated, no collectives.

Per-core dataflow (two phases, chunk = 512 tokens = one window-row):
  Phase AB (ACT table: exp only):
    x -> LN1 (token-major, rsqrt via magic-constant Newton on DVE)
      -> PE-transpose -> h_win (channel-major, window-token order)
      -> QKV (q,k channel-major / v token-major) -> windowed attention
      (scores_T = k^T q in PE quadrant strips, softmax as exp * host-built
      exp(bias+mask), normalization deferred past AV) -> proj -> scatter
      proj output (window+roll reverse) to DRAM scratch yp.
  Phase C (ACT table: gelu_apprx_tanh):
    y = x + yp (contiguous reads) -> LN2 (magic rsqrt) -> PE-transpose
      -> FC1 -> gelu -> FC2 -> out = y + mlp.
"""

import sys

sys.path.insert(0, "/opt/trn_rl_repo")

import os
import numpy as np
import ml_dtypes

import concourse.bass as bass
import concourse.mybir as mybir
import concourse.tile as tile
from concourse import bacc
from concourse.bass_utils import run_bass_kernel_spmd
from concourse.masks import make_identity

F32 = mybir.dt.float32
BF16 = mybir.dt.bfloat16
I32 = mybir.dt.int32
AF = mybir.ActivationFunctionType
ALU = mybir.AluOpType

B, H, W, C = 32, 64, 64, 256
WS, SS, NH = 8, 4, 8
HD = C // NH          # 32
N = WS * WS           # 64 tokens / window
NWX = W // WS         # 8 windows per row
NR = H // WS          # 8 window rows
G = 4                 # images per core
EPS = 1e-5
MAGIC = 0x5F3759DF

# number of images processed per core in the device graph (dev knob)
N_IMG = int(os.environ.get("KERNEL_N_IMG", str(G)))
# decompose gelu via Tanh (CoreSim lacks Gelu_apprx_tanh); HW uses the LUT
SIM_GELU = bool(int(os.environ.get("KERNEL_SIM_GELU", "0")))
# disable PE sub-array packing (debug knob)
NO_TILEPOS = bool(int(os.environ.get("KERNEL_NO_TILEPOS", "0")))
# run only a subset of the graph (debug): ALL | AB | C
PHASES = os.environ.get("KERNEL_PHASES", "ALL")
# stage limit inside AB (debug): 99=all, 1=ln, 2=transp, 3=qkv, 4=scores/exp, 5=av
AB_STAGE = int(os.environ.get("KERNEL_AB_STAGE", "99"))
# debug: force all scores outputs to column-strip 0 (wrong results)
SC_COL0 = bool(int(os.environ.get("KERNEL_SC_COL0", "0")))


# ----------------------------------------------------------------------------
# host-side constant prep
# ----------------------------------------------------------------------------

def _rel_pos_index():
    coords = np.stack(np.meshgrid(np.arange(WS), np.arange(WS), indexing="ij"))
    cf = coords.reshape(2, -1)
    rel = (cf[:, :, None] - cf[:, None, :]).transpose(1, 2, 0).copy()
    rel[:, :, 0] += WS - 1
    rel[:, :, 1] += WS - 1
    rel[:, :, 0] *= 2 * WS - 1
    return rel.sum(-1)


def _shift_mask():
    img = np.zeros((H, W))
    cnt = 0
    for hs in (slice(0, -WS), slice(-WS, -SS), slice(-SS, None)):
        for ws_ in (slice(0, -WS), slice(-WS, -SS), slice(-SS, None)):
            img[hs, ws_] = cnt
            cnt += 1
    mw = img.reshape(H // WS, WS, W // WS, WS).transpose(0, 2, 1, 3).reshape(-1, N)
    diff = mw[:, None, :] - mw[:, :, None]
    return np.where(diff != 0, -100.0, 0.0).astype(np.float32)  # [nW, N, N]


def host_prep(inputs):
    """Build all device-constant arrays from the raw weights (numpy)."""
    f32 = lambda a: np.ascontiguousarray(a, dtype=np.float32)
    bf = lambda a: np.ascontiguousarray(a.astype(ml_dtypes.bfloat16))

    qkv_w = f32(inputs["qkv_w"])      # [768, 256]
    qkv_b = f32(inputs["qkv_b"])      # [768]
    g1 = f32(inputs["norm1_g"]); b1 = f32(inputs["norm1_b"])
    g2 = f32(inputs["norm2_g"]); b2 = f32(inputs["norm2_b"])
    rpb = f32(inputs["rpb_table"])    # [225, 8]
    proj_w = f32(inputs["proj_w"]); proj_b = f32(inputs["proj_b"])
    fc1_w = f32(inputs["fc1_w"]); fc1_b = f32(inputs["fc1_b"])
    fc2_w = f32(inputs["fc2_w"]); fc2_b = f32(inputs["fc2_b"])

    # fold LN1 affine into qkv: LN(x) = z*g1 + b1  =>  qkv = z@(g1*W).T + (W@b1 + qkv_b)
    W1 = qkv_w * g1[None, :]
    bq = qkv_b + qkv_w @ b1                     # [768]
    W1 = W1.reshape(3, NH, HD, C)               # row o = s*256 + h*32 + d
    bq = bq.reshape(3, NH, HD)
    scale = HD ** -0.5
    Wq = W1[0] * scale; bqq = bq[0] * scale     # fold softmax scale into q
    Wk = W1[1]; bqk = bq[1]
    Wv = W1[2]; bqv = bq[2]

    # qk weights transposed, m-blocks: [q h0-3][q h4-7][k h0-3][k h4-7]
    blocks = [Wq[0:4].reshape(128, C), Wq[4:8].reshape(128, C),
              Wk[0:4].reshape(128, C), Wk[4:8].reshape(128, C)]
    wqkT = np.stack([blk.T for blk in blocks], axis=1)      # [256, 4, 128]
    wqkT = wqkT.reshape(2, 128, 4, 128)
    b_qk = np.stack([bqq[0:4].reshape(-1), bqq[4:8].reshape(-1),
                     bqk[0:4].reshape(-1), bqk[4:8].reshape(-1)], axis=1)  # [128, 4]

    # v: moving rhs = WvT [2, 128(c), 256(h*32+d)]
    wvT = Wv.reshape(C, C).T.reshape(2, 128, 256).copy()
    b_v = bqv.reshape(C)

    # exp(bias + mask), transposed to [k, q], replicated across both 64-halves
    rpi = _rel_pos_index()
    bias_qk = rpb[rpi.reshape(-1)].reshape(N, N, NH).transpose(2, 0, 1)  # [h, q, k]
    mask = _shift_mask().reshape(NR, NWX, N, N)
    classes = [(0, 0), (0, NWX - 1), (NR - 1, 0), (NR - 1, NWX - 1)]
    for wy in range(NR):
        for wx in range(NWX):
            cy = NR - 1 if wy == NR - 1 else 0
            cx = NWX - 1 if wx == NWX - 1 else 0
            assert np.array_equal(mask[wy, wx], mask[cy, cx])
    expB = np.zeros((4, 128, NH, N), dtype=np.float32)  # [cls, k(2x64), h, q]
    for ci, (cy, cx) in enumerate(classes):
        ebT = np.exp(bias_qk + mask[cy, cx][None]).transpose(2, 0, 1)  # [k, h, q]
        expB[ci, 0:64] = ebT
        expB[ci, 64:128] = ebT

    projT = proj_w.T.reshape(2, 128, 256).copy()
    b_p = proj_b

    # fold LN2 affine into fc1
    W3 = fc1_w * g2[None, :]
    bf1 = fc1_b + fc1_w @ b2
    fc1T = W3.T.reshape(2, 128, 1024).copy()
    b_f1 = bf1.reshape(8, 128).T.copy()          # [128, 8] per-partition
    fc2T = fc2_w.T.reshape(8, 128, 256).copy()
    b_f2 = fc2_b

    return {
        "wqkT": bf(wqkT), "b_qk": f32(b_qk),
        "wvT": bf(wvT), "b_v": f32(b_v),
        "expB": bf(expB),
        "projT": bf(projT), "b_p": f32(b_p),
        "fc1T": bf(fc1T), "b_f1": f32(b_f1),
        "fc2T": bf(fc2T), "b_f2": f32(b_f2),
    }


# ----------------------------------------------------------------------------
# device helpers
# ----------------------------------------------------------------------------

def _magic_rsqrt(nc, sp, out, var_ap, magic_t, n_iter=3):
    """out = 1/sqrt(var + EPS) on DVE only (no ACT table).

    out/var_ap: [128, F] f32 APs (small F). magic_t: [128, 1] i32 const."""
    shape = list(var_ap.shape)
    va = sp.tile(shape, F32, tag="rs_va", name="rs_va")
    nc.vector.tensor_scalar(va[:], var_ap, float(EPS), None, op0=ALU.add)
    xi = sp.tile(shape, I32, tag="rs_xi", name="rs_xi")
    nc.vector.tensor_scalar(xi[:], va[:].bitcast(I32), 1, None,
                            op0=ALU.logical_shift_right)
    nc.vector.tensor_tensor(xi[:], magic_t.to_broadcast(shape), xi[:],
                            ALU.subtract)
    x = xi[:].bitcast(F32)
    t = sp.tile(shape, F32, tag="rs_t", name="rs_t")
    for _ in range(n_iter):
        nc.vector.tensor_tensor(t[:], x, x, ALU.mult)
        nc.vector.tensor_tensor(t[:], va[:], t[:], ALU.mult)
        nc.vector.tensor_scalar(t[:], t[:], -0.5, 1.5, op0=ALU.mult, op1=ALU.add)
        nc.vector.tensor_tensor(out, x, t[:], ALU.mult)
        x = out


# ----------------------------------------------------------------------------
# device graph
# ----------------------------------------------------------------------------

def build(has_bv, has_bp, has_bf2):
    nc = bacc.Bacc("TRN2", target_bir_lowering=False, debug=False, num_devices=8)

    x = nc.dram_tensor("x", [G, H, W, C], F32, kind="ExternalInput").ap()
    wqkT_d = nc.dram_tensor("wqkT", [2, 128, 4, 128], BF16, kind="ExternalInput").ap()
    b_qk_d = nc.dram_tensor("b_qk", [128, 4], F32, kind="ExternalInput").ap()
    wvT_d = nc.dram_tensor("wvT", [2, 128, 256], BF16, kind="ExternalInput").ap()
    b_v_d = nc.dram_tensor("b_v", [256], F32, kind="ExternalInput").ap()
    expB_d = nc.dram_tensor("expB", [4, 128, NH, N], BF16, kind="ExternalInput").ap()
    projT_d = nc.dram_tensor("projT", [2, 128, 256], BF16, kind="ExternalInput").ap()
    b_p_d = nc.dram_tensor("b_p", [256], F32, kind="ExternalInput").ap()
    fc1T_d = nc.dram_tensor("fc1T", [2, 128, 1024], BF16, kind="ExternalInput").ap()
    b_f1_d = nc.dram_tensor("b_f1", [128, 8], F32, kind="ExternalInput").ap()
    fc2T_d = nc.dram_tensor("fc2T", [8, 128, 256], BF16, kind="ExternalInput").ap()
    b_f2_d = nc.dram_tensor("b_f2", [256], F32, kind="ExternalInput").ap()
    out = nc.dram_tensor("out", [G, H * W, C], F32, kind="ExternalOutput").ap()

    with tile.TileContext(nc) as tc:
        with tc.tile_pool(name="const", bufs=1) as cpool, \
             tc.tile_pool(name="work", bufs=3) as wp, \
             tc.tile_pool(name="stats", bufs=4) as sp, \
             tc.tile_pool(name="psmm", bufs=4, space="PSUM") as psmm, \
             tc.tile_pool(name="pssc", bufs=4, space="PSUM") as pssc, \
             tc.tile_pool(name="dram", bufs=1, space="DRAM") as dpool:

            # ---------------- resident constants ----------------
            wqkT = cpool.tile([128, 2, 4, 128], BF16)
            nc.sync.dma_start(wqkT[:], wqkT_d.rearrange("k c m o -> c k m o"))
            b_qk = cpool.tile([128, 4], F32)
            nc.sync.dma_start(b_qk[:], b_qk_d[:])
            wvT = cpool.tile([128, 2, 256], BF16)
            nc.sync.dma_start(wvT[:], wvT_d.rearrange("k c o -> c k o"))
            expB = cpool.tile([128, 4, NH, N], BF16)
            nc.sync.dma_start(expB[:], expB_d.rearrange("s c h q -> c s h q"))
            projT = cpool.tile([128, 2, 256], BF16)
            nc.sync.dma_start(projT[:], projT_d.rearrange("k c o -> c k o"))
            fc1T = cpool.tile([128, 2, 1024], BF16)
            nc.sync.dma_start(fc1T[:], fc1T_d.rearrange("k c o -> c k o"))
            b_f1 = cpool.tile([128, 8], F32)
            nc.sync.dma_start(b_f1[:], b_f1_d[:])
            fc2T = cpool.tile([128, 8, 256], BF16)
            nc.sync.dma_start(fc2T[:], fc2T_d.rearrange("k c o -> c k o"))

            ident = cpool.tile([128, 128], BF16)
            make_identity(nc, ident[:])
            ones_row = cpool.tile([1, 128], BF16)
            nc.vector.memset(ones_row[:], 1.0)
            magic_t = cpool.tile([128, 1], I32)
            nc.vector.memset(magic_t[:], MAGIC)

            bias_rows = cpool.tile([1, 3, 256], F32)  # [b_v | b_p | b_f2]
            nc.sync.dma_start(bias_rows[:, 0, :], b_v_d[None, :])
            nc.sync.dma_start(bias_rows[:, 1, :], b_p_d[None, :])
            nc.sync.dma_start(bias_rows[:, 2, :], b_f2_d[None, :])
            bias_rows_bf = cpool.tile([1, 3, 256], BF16)
            nc.vector.tensor_copy(bias_rows_bf[:], bias_rows[:])

            yp_dram = dpool.tile([G, H, W, C], F32)

            # ================= phase AB =================
            for g in range(N_IMG if PHASES in ("ALL", "AB") else 0):
                for wy in range(NR):
                    # ---- load x rows (raw token order) ----
                    x_tm = wp.tile([128, 4, 256], F32, tag="x_tm")
                    for (js, jn, rs) in (
                            [(0, 4, WS * wy + SS)] if wy < NR - 1
                            else [(0, 2, H - SS), (2, 2, 0)]):
                        nc.sync.dma_start(
                            x_tm[:, js:js + jn, :],
                            x[g, rs:rs + 2 * jn].rearrange(
                                "(j two) w c -> (two w) j c", two=2))

                    # ---- LN1 stats + normalize (token-major) ----
                    st = sp.tile([128, 4, 6], F32, tag="st")
                    mv = sp.tile([128, 4, 2], F32, tag="mv")
                    for j in range(4):
                        nc.vector.bn_stats(st[:, j, :], x_tm[:, j, :])
                        nc.vector.bn_aggr(mv[:, j, :], st[:, j, :])
                    r1 = sp.tile([128, 4], F32, tag="r1")
                    _magic_rsqrt(nc, sp, r1[:], mv[:, :, 1], magic_t[:])
                    h_tm = wp.tile([128, 4, 256], BF16, tag="h_tm")
                    for j in range(4):
                        nc.vector.tensor_scalar(
                            h_tm[:, j, :], x_tm[:, j, :],
                            mv[:, j, 0:1], r1[:, j:j + 1],
                            op0=ALU.subtract, op1=ALU.mult)

                    if AB_STAGE < 2:
                        continue
                    # ---- transpose to channel-major, window-token order ----
                    # h_win free index = wx*64 + py*8 + px (window-gathered)
                    h_win = wp.tile([128, 2, 512], BF16, tag="h_win")
                    for j in range(4):
                        for t in range(2):
                            ptr = psmm.tile([128, 128], BF16, tag="mm")
                            nc.tensor.transpose(
                                ptr[:], h_tm[:, j, 128 * t:128 * (t + 1)], ident[:])
                            # ptr free = (two, col); shifted col -> (wx, px)
                            pv3 = ptr[:].rearrange("c (two w) -> c two w", two=2)
                            dst = h_win[:, t, :].rearrange(
                                "c (wx py px) -> c wx py px", py=8, px=8)
                            nc.vector.tensor_copy(
                                dst[:, 0:7, 2 * j:2 * j + 2, :],
                                pv3[:, :, SS:SS + 56].rearrange(
                                    "c two (wx px) -> c wx two px", px=8))
                            nc.vector.tensor_copy(
                                dst[:, 7, 2 * j:2 * j + 2, 0:4],
                                pv3[:, :, 64 - SS:64])
                            nc.vector.tensor_copy(
                                dst[:, 7, 2 * j:2 * j + 2, 4:8],
                                pv3[:, :, 0:SS])

                    if AB_STAGE < 3:
                        continue
                    # ---- qkv: q,k channel-major (weights stationary) ----
                    qk_sb = wp.tile([128, 4, 512], BF16, tag="qk_sb")
                    for m in range(4):
                        pqk = psmm.tile([128, 512], F32, tag="mm")
                        for kk in range(2):
                            nc.tensor.matmul(pqk[:], wqkT[:, kk, m, :], h_win[:, kk, :],
                                             start=(kk == 0), stop=(kk == 1))
                        nc.vector.tensor_scalar(
                            qk_sb[:, m, :], pqk[:], b_qk[:, m:m + 1], None, op0=ALU.add)

                    # ---- v token-major (activation stationary) ----
                    v_aug = wp.tile([128, 4, NH, 33], BF16, tag="v_aug")
                    for tt in range(4):
                        pv = psmm.tile([128, 256], F32, tag="mm")
                        for kk in range(2):
                            nc.tensor.matmul(pv[:], h_win[:, kk, 128 * tt:128 * (tt + 1)],
                                             wvT[:, kk, :],
                                             start=(kk == 0), stop=(kk == 1))
                        if has_bv:
                            nc.tensor.matmul(pv[:], ones_row[:, 0:128],
                                             bias_rows_bf[:, 0, :],
                                             start=False, stop=True)
                        nc.vector.tensor_copy(
                            v_aug[:, tt, :, 0:32],
                            pv[:].rearrange("p (h d) -> p h d", h=NH))
                    nc.vector.memset(v_aug[:, :, :, 32:33], 1.0)

                    if AB_STAGE < 4:
                        continue
                    # ---- scores_T = k^T q per (w, h) in PE quadrant strips ----
                    # Per-strip psum tiles: concurrently-packed strip matmuls
                    # must write distinct PSUM banks (same-bank pairs from
                    # different row strips crash the PE).
                    # sc[s] layout: [128 = 64*(w&1)+k, mq, wpair, q]
                    sc = [pssc.tile([128, 2, 4, N], F32, tag="sc", name=f"sc{_i}")
                          for _i in range(4)]
                    for w in range(8):
                        for h in range(NH):
                            s = h % 4
                            mq = 0 if h < 4 else 1
                            p0 = 64 * (w & 1)
                            nc.tensor.matmul(
                                sc[s][p0:p0 + 64, mq, w // 2, :],
                                qk_sb[32 * s:32 * s + 32, 2 + mq, N * w:N * (w + 1)],
                                qk_sb[32 * s:32 * s + 32, mq, N * w:N * (w + 1)],
                                start=True, stop=True,
                                tile_position=(32 * s, p0))
                    if AB_STAGE < 5:
                        continue
                    # ---- exp, then * exp(bias+mask) ----
                    E = wp.tile([128, NH, 4, N], BF16, tag="E")
                    E5 = E[:].rearrange("c (mq s) wp q -> c mq s wp q", s=4)
                    for s in range(4):
                        nc.scalar.activation(E5[:, :, s, :, :], sc[s][:], AF.Exp)
                    c0 = 0 if wy < NR - 1 else 2
                    c1 = 1 if wy < NR - 1 else 3
                    if AB_STAGE < 6:
                        continue
                    nc.vector.tensor_tensor(
                        E[:, :, 0:3, :], E[:, :, 0:3, :],
                        expB[:, c0, :, None, :].to_broadcast([128, NH, 3, N]),
                        ALU.mult)
                    nc.vector.tensor_tensor(
                        E[0:64, :, 3, :], E[0:64, :, 3, :],
                        expB[0:64, c0, :, :], ALU.mult)
                    nc.vector.tensor_tensor(
                        E[64:128, :, 3, :], E[64:128, :, 3, :],
                        expB[64:128, c1, :, :], ALU.mult)

                    if AB_STAGE < 7:
                        continue
                    # ---- AV + normalize + transpose, per w-pair ----
                    # attn_raw collects the transposed attention output with
                    # tokens permuted back to RAW chunk order (row-pair, col):
                    # window token (w, py, px) -> col (8w+4+px)%64 of row py.
                    attn_raw = wp.tile([128, 2, 512], BF16, tag="attn_raw")
                    for tt in range(4):
                        av = psmm.tile([128, NH, 33], F32, tag="mm")
                        for dw in range(2):
                            p0 = 64 * dw
                            for h in range(NH):
                                nc.tensor.matmul(
                                    av[p0:p0 + 64, h, :],
                                    E[p0:p0 + 64, h, tt, :],
                                    v_aug[p0:p0 + 64, tt, h, :],
                                    start=True, stop=True,
                                    **({} if NO_TILEPOS else
                                       dict(tile_position=(p0, p0))))
                        rs_ = sp.tile([128, NH], F32, tag="rs")
                        nc.vector.reciprocal(rs_[:], av[:, :, 32])
                        a_tm = wp.tile([128, 256], BF16, tag="a_tm")
                        nc.vector.tensor_tensor(
                            a_tm[:].rearrange("p (h d) -> p h d", h=NH),
                            av[:, :, 0:32],
                            rs_[:, :, None].to_broadcast([128, NH, 32]),
                            ALU.mult)
                        for ct in range(2):
                            ptr = psmm.tile([128, 128], BF16, tag="mm")
                            nc.tensor.transpose(
                                ptr[:], a_tm[:, 128 * ct:128 * (ct + 1)], ident[:])
                            # ptr free = (dw, py, px); place px-runs at raw cols
                            p3 = ptr[:].rearrange("c (dw py px) -> c dw py px",
                                                  dw=2, px=8)
                            d3 = attn_raw[:, ct, :].rearrange(
                                "c (py col) -> c py col", col=64)
                            for dw in range(2):
                                w = 2 * tt + dw
                                if w < NWX - 1:
                                    nc.vector.tensor_copy(
                                        d3[:, :, WS * w + SS:WS * w + SS + 8],
                                        p3[:, dw, :, :])
                                else:
                                    nc.vector.tensor_copy(
                                        d3[:, :, W - SS:W],
                                        p3[:, dw, :, 0:SS])
                                    nc.vector.tensor_copy(
                                        d3[:, :, 0:SS],
                                        p3[:, dw, :, SS:WS])

                    # ---- proj on raw-ordered tokens + contiguous store ----
                    for j in range(4):
                        py_ = psmm.tile([128, 256], F32, tag="mm")
                        for kk in range(2):
                            nc.tensor.matmul(
                                py_[:], attn_raw[:, kk, 128 * j:128 * (j + 1)],
                                projT[:, kk, :],
                                start=(kk == 0), stop=(kk == 1))
                        if has_bp:
                            nc.tensor.matmul(py_[:], ones_row[:, 0:128],
                                             bias_rows_bf[:, 1, :],
                                             start=False, stop=True)
                        yp_sb = wp.tile([128, 256], F32, tag="yp_sb")
                        nc.vector.tensor_copy(yp_sb[:], py_[:])
                        r = (WS * wy + SS + 2 * j) % H
                        nc.sync.dma_start(
                            yp_dram[g, r:r + 2, :, :], yp_sb[:])

            # ================= phase C =================
            tc.strict_bb_all_engine_barrier()
            for g in range(N_IMG if PHASES in ("ALL", "C") else 0):
                for ri in range(8):
                    x_c = wp.tile([128, 4, 256], F32, tag="x_c")
                    nc.sync.dma_start(
                        x_c[:],
                        x[g, 8 * ri:8 * ri + 8].rearrange(
                            "(j two) w c -> (two w) j c", two=2))
                    yp_c = wp.tile([128, 4, 256], F32, tag="yp_c")
                    nc.sync.dma_start(
                        yp_c[:],
                        yp_dram[g, 8 * ri:8 * ri + 8].rearrange(
                            "(j two) w c -> (two w) j c", two=2))
                    y_sb = wp.tile([128, 4, 256], F32, tag="y_sb")
                    nc.vector.tensor_tensor(y_sb[:], x_c[:], yp_c[:], ALU.add)

                    st = sp.tile([128, 4, 6], F32, tag="st")
                    mv = sp.tile([128, 4, 2], F32, tag="mv")
                    for j in range(4):
                        nc.vector.bn_stats(st[:, j, :], y_sb[:, j, :])
                        nc.vector.bn_aggr(mv[:, j, :], st[:, j, :])
                    r2 = sp.tile([128, 4], F32, tag="r1")
                    _magic_rsqrt(nc, sp, r2[:], mv[:, :, 1], magic_t[:])

                    z = wp.tile([128, 4, 256], BF16, tag="z")
                    for j in range(4):
                        nc.vector.tensor_scalar(
                            z[:, j, :], y_sb[:, j, :],
                            mv[:, j, 0:1], r2[:, j:j + 1],
                            op0=ALU.subtract, op1=ALU.mult)
                    z_cm = wp.tile([128, 2, 512], BF16, tag="z_cm")
                    for j in range(4):
                        for t in range(2):
                            ptr = psmm.tile([128, 128], BF16, tag="mm")
                            nc.tensor.transpose(
                                ptr[:], z[:, j, 128 * t:128 * (t + 1)], ident[:])
                            nc.vector.tensor_copy(
                                z_cm[:, t, 128 * j:128 * (j + 1)], ptr[:])
                    for sub in range(2):
                        h1 = wp.tile([128, 8, 256], BF16, tag="h1")
                        for m in range(8):
                            p1 = psmm.tile([128, 256], F32, tag="mm")
                            for kk in range(2):
                                nc.tensor.matmul(
                                    p1[:], fc1T[:, kk, 128 * m:128 * (m + 1)],
                                    z_cm[:, kk, 256 * sub:256 * (sub + 1)],
                                    start=(kk == 0), stop=(kk == 1))
                            if not SIM_GELU:
                                nc.scalar.activation(
                                    h1[:, m, :], p1[:],
                                    AF.Gelu_apprx_tanh, bias=b_f1[:, m:m + 1])
                            else:
                                # 0.5*u*(1+tanh(0.7978845608*(u+0.044715*u^3)))
                                ub = wp.tile([128, 256], F32, tag="g_ub", name="g_ub")
                                nc.vector.tensor_scalar(
                                    ub[:], p1[:], b_f1[:, m:m + 1], None, op0=ALU.add)
                                t2 = wp.tile([128, 256], F32, tag="g_t2", name="g_t2")
                                nc.vector.tensor_tensor(t2[:], ub[:], ub[:], ALU.mult)
                                nc.vector.tensor_tensor(t2[:], t2[:], ub[:], ALU.mult)
                                nc.vector.tensor_scalar(
                                    t2[:], t2[:], 0.044715, None, op0=ALU.mult)
                                nc.vector.tensor_tensor(t2[:], t2[:], ub[:], ALU.add)
                                nc.scalar.activation(t2[:], t2[:], AF.Tanh,
                                                     scale=0.7978845608028654)
                                nc.vector.tensor_scalar(
                                    t2[:], t2[:], 1.0, 0.5, op0=ALU.add, op1=ALU.mult)
                                nc.vector.tensor_tensor(
                                    h1[:, m, :], t2[:], ub[:], ALU.mult)
                        for i in range(2):
                            tt = 2 * sub + i
                            p2 = psmm.tile([128, 256], F32, tag="mm")
                            for kk in range(8):
                                nc.tensor.matmul(
                                    p2[:], h1[:, kk, 128 * i:128 * (i + 1)],
                                    fc2T[:, kk, :],
                                    start=(kk == 0), stop=(kk == 7))
                            if has_bf2:
                                nc.tensor.matmul(p2[:], ones_row[:, 0:128],
                                                 bias_rows_bf[:, 2, :],
                                                 start=False, stop=True)
                            o_sb = wp.tile([128, 256], F32, tag="o_sb")
                            nc.vector.tensor_tensor(
                                o_sb[:], y_sb[:, tt, :], p2[:], ALU.add)
                            nc.sync.dma_start(
                                out[g, 512 * ri + 128 * tt:512 * ri + 128 * (tt + 1), :],
                                o_sb[:])

    nc.compile()
    return nc


# ----------------------------------------------------------------------------
# public entry
# ----------------------------------------------------------------------------

_NC_CACHE = {}


def kernel(**inputs):
    consts = host_prep(inputs)
    key = (bool(np.any(consts["b_v"])), bool(np.any(consts["b_p"])),
           bool(np.any(consts["b_f2"])))
    if key not in _NC_CACHE:
        _NC_CACHE[key] = build(*key)
    nc = _NC_CACHE[key]

    x = np.ascontiguousarray(inputs["x"], dtype=np.float32).reshape(B, H, W, C)
    in_maps = []
    for core in range(8):
        m = {"x": np.ascontiguousarray(x[G * core:G * (core + 1)])}
        m.update(consts)
        in_maps.append(m)
    res = run_bass_kernel_spmd(nc, in_maps, list(range(8)))
    outs = [res.results[i]["out"] for i in range(8)]
    return np.concatenate(outs, axis=0).astype(np.float32)
